# revision 7
# baseline (speedup 1.0000x reference)
"""Trainium2 Bass kernel for nn_MixtralOfExpertsLayer (MoE, top-2 of 8 experts).

Sharding: token-parallel over 8 NeuronCores. Each core owns 1024 tokens
end-to-end (router + all-expert FFN + weighted combine), so no collectives
are needed; the host only splits x and concatenates the per-core outputs.

Per-core pipeline (T-formulation: activations kept as [feature, token]):
  - gate logits in exact fp32 on the PE, top-2 via vector max/max_index,
    renormalized weights via the sigmoid identity g2 = sigmoid(l2-l1).
  - dense FFN over all 8 experts in float32r (full-rate PE), scaled by the
    masked gate weights, accumulated in SBUF.
  - PE-transpose back to [token, feature] and DMA out.
"""

import sys

import numpy as np

sys.path.insert(0, "/opt/trn_rl_repo")

from concourse import bacc, bass, mybir  # noqa: E402
import concourse.tile as tile  # noqa: E402
from concourse.bass_utils import run_bass_kernel_spmd  # noqa: E402
from concourse.masks import make_identity  # noqa: E402

B, T, D, H, O, E = 4, 2048, 1024, 2048, 1024, 8
N_CORES = 8
NTOK = (B * T) // N_CORES  # 1024 tokens per core
P = 128
KD = D // P   # 8 contraction tiles for D
MH = H // P   # 16 partition tiles for H
MO = O // P   # 8 partition tiles for O
TM = NTOK // P  # 8 token tiles per core
NCH = 512     # matmul moving free-dim (one PSUM bank in fp32)
NNC = NTOK // NCH  # 2

f32 = mybir.dt.float32
f32r = mybir.dt.float32r
u32 = mybir.dt.uint32
AF = mybir.ActivationFunctionType
ALU = mybir.AluOpType

_CACHE: dict = {}


def _build():
    nc = bacc.Bacc("TRN2", target_bir_lowering=False, debug=False,
                   num_devices=N_CORES)
    xt = nc.declare_dram_parameter("xt", [D, NTOK], f32r, isOutput=False)
    xtg = nc.declare_dram_parameter("xtg", [D, NTOK], f32, isOutput=False)
    wg = nc.declare_dram_parameter("wg", [D, E], f32, isOutput=False)
    bgb = nc.declare_dram_parameter("bgb", [P, E], f32, isOutput=False)
    w1 = nc.declare_dram_parameter("w1", [E, D, H], f32r, isOutput=False)
    b1 = nc.declare_dram_parameter("b1", [E, H, 1], f32, isOutput=False)
    w2 = nc.declare_dram_parameter("w2", [E, H, O], f32r, isOutput=False)
    b2 = nc.declare_dram_parameter("b2", [O, 1], f32, isOutput=False)
    y = nc.declare_dram_parameter("y", [NTOK, O], f32, isOutput=True)

    with tile.TileContext(nc) as tc:
        with (
            tc.tile_pool(name="const", bufs=1) as constp,
            tc.tile_pool(name="res", bufs=1) as resp,
            tc.tile_pool(name="wstr", bufs=3) as wp,
            tc.tile_pool(name="gate", bufs=2) as gp,
            tc.tile_pool(name="tmp", bufs=3) as tmpp,
            tc.tile_pool(name="outs", bufs=2) as outp,
            tc.tile_pool(name="psmm", bufs=4, space="PSUM") as psmm,
            tc.tile_pool(name="psg", bufs=1, space="PSUM") as psg,
            tc.tile_pool(name="pstr", bufs=2, space="PSUM") as pstr,
        ):
            # ---- constants ----
            idn = constp.tile([P, P], f32, tag="idn")
            make_identity(nc, idn[:])
            iot = constp.tile([P, E], f32, tag="iot")
            nc.gpsimd.iota(iot[:], pattern=[[1, E]], base=0,
                           channel_multiplier=0,
                           allow_small_or_imprecise_dtypes=True)
            bgsb = constp.tile([P, E], f32, tag="bgsb")
            nc.sync.dma_start(out=bgsb[:], in_=bgb[:])
            b2sb = constp.tile([P, MO], f32, tag="b2sb")
            nc.sync.dma_start(
                out=b2sb[:],
                in_=b2.rearrange("(om p) one -> p (om one)", p=P))
            wgsb = constp.tile([P, KD * E], f32, tag="wgsb")
            nc.sync.dma_start(
                out=wgsb[:].rearrange("p (kd e) -> p kd e", e=E),
                in_=wg.rearrange("(kd p) e -> p kd e", p=P))

            # ---- resident activations: x^T in f32r for the FFN ----
            xtr = []
            for kd in range(KD):
                t = resp.tile([P, NTOK], f32r, tag=f"xtr{kd}", name=f"xtr{kd}")
                nc.sync.dma_start(out=t[:], in_=xt[kd * P:(kd + 1) * P, :])
                xtr.append(t)

            # ---- gate: logits, top-2, renormalized weights ----
            # gtrow[e][0, tok]: per-expert gate weight row (0 if not routed)
            gtrow = resp.tile([1, E * NTOK], f32, tag="gtrow",
                              name="gtrow")
            for tm in range(TM):
                ts = slice(tm * P, (tm + 1) * P)
                pg = psg.tile([P, E], f32, tag="pg")
                for kd in range(KD):
                    xg = gp.tile([P, P], f32, tag="xg")
                    nc.sync.dma_start(
                        out=xg[:], in_=xtg[kd * P:(kd + 1) * P, ts])
                    nc.tensor.matmul(
                        pg[:], lhsT=xg[:],
                        rhs=wgsb[:, kd * E:(kd + 1) * E],
                        start=(kd == 0), stop=(kd == KD - 1))
                lg = gp.tile([P, E], f32, tag="lg")
                nc.vector.tensor_add(out=lg[:], in0=pg[:], in1=bgsb[:])
                vm = gp.tile([P, E], f32, tag="vm")
                nc.vector.max(vm[:], lg[:])
                vi = gp.tile([P, E], u32, tag="vi")
                nc.vector.max_index(vi[:], vm[:], lg[:])
                vif = gp.tile([P, E], f32, tag="vif")
                nc.vector.tensor_copy(out=vif[:], in_=vi[:])
                dlt = gp.tile([P, 1], f32, tag="dlt")
                nc.vector.tensor_sub(dlt[:], vm[:, 1:2], vm[:, 0:1])
                g2 = gp.tile([P, 1], f32, tag="g2")
                nc.scalar.activation(out=g2[:], in_=dlt[:], func=AF.Sigmoid)
                g1 = gp.tile([P, 1], f32, tag="g1")
                nc.vector.tensor_scalar(g1[:], g2[:], -1.0, 1.0,
                                        ALU.mult, ALU.add)
                m1 = gp.tile([P, E], f32, tag="m1")
                nc.vector.tensor_tensor(
                    out=m1[:], in0=vif[:, 0:1].to_broadcast([P, E]),
                    in1=iot[:], op=ALU.is_equal)
                m2 = gp.tile([P, E], f32, tag="m2")
                nc.vector.tensor_tensor(
                    out=m2[:], in0=vif[:, 1:2].to_broadcast([P, E]),
                    in1=iot[:], op=ALU.is_equal)
                t1 = gp.tile([P, E], f32, tag="t1")
                nc.vector.tensor_tensor(
                    out=t1[:], in0=m1[:], in1=g1[:].to_broadcast([P, E]),
                    op=ALU.mult)
                t2 = gp.tile([P, E], f32, tag="t2")
                nc.vector.tensor_tensor(
                    out=t2[:], in0=m2[:], in1=g2[:].to_broadcast([P, E]),
                    op=ALU.mult)
                gv = gp.tile([P, E], f32, tag="gv")
                nc.vector.tensor_add(out=gv[:], in0=t1[:], in1=t2[:])
                for e in range(E):
                    pt1 = pstr.tile([1, P], f32, tag="tr", name="pt1")
                    nc.tensor.transpose(out=pt1[:], in_=gv[:, e:e + 1],
                                        identity=idn[:])
                    nc.vector.tensor_copy(
                        out=gtrow[:, e * NTOK + tm * P:e * NTOK + (tm + 1) * P],
                        in_=pt1[:])

            # ---- dense FFN over experts, f32r, gate-scaled accumulate ----
            acc = [resp.tile([P, NTOK], f32, tag=f"acc{om}", name=f"acc{om}")
                   for om in range(MO)]
            ht = [resp.tile([P, NTOK], f32r, tag=f"ht{hm}", name=f"ht{hm}")
                  for hm in range(MH)]
            for e in range(E):
                gtb = tmpp.tile([P, NTOK], f32, tag="gtb", name="gtb", bufs=2)
                nc.gpsimd.partition_broadcast(
                    gtb[:], gtrow[:, e * NTOK:(e + 1) * NTOK])
                for hm in range(MH):
                    w1sb = wp.tile([P, KD * P], f32r, tag="w1sb", bufs=2)
                    nc.sync.dma_start(
                        out=w1sb[:].rearrange("p (kd h) -> p kd h", h=P),
                        in_=w1[e, :, hm * P:(hm + 1) * P]
                        .rearrange("(kd p) h -> p kd h", p=P))
                    b1c = tmpp.tile([P, 1], f32, tag="b1c")
                    nc.sync.dma_start(
                        out=b1c[:], in_=b1[e, hm * P:(hm + 1) * P, :])
                    for nn in range(NNC):
                        ns = slice(nn * NCH, (nn + 1) * NCH)
                        ph = psmm.tile([P, NCH], f32, tag="mm")
                        for kd in range(KD):
                            nc.tensor.matmul(
                                ph[:], lhsT=w1sb[:, kd * P:(kd + 1) * P],
                                rhs=xtr[kd][:, ns],
                                start=(kd == 0), stop=(kd == KD - 1))
                        nc.scalar.activation(
                            out=ht[hm][:, ns], in_=ph[:], func=AF.Relu,
                            bias=b1c[:])
                for om in range(MO):
                    w2sb = wp.tile([P, MH * P], f32r, tag="w2sb", bufs=2)
                    nc.sync.dma_start(
                        out=w2sb[:].rearrange("p (kh o) -> p kh o", o=P),
                        in_=w2[e, :, om * P:(om + 1) * P]
                        .rearrange("(kh p) o -> p kh o", p=P))
                    for nn in range(NNC):
                        ns = slice(nn * NCH, (nn + 1) * NCH)
                        po = psmm.tile([P, NCH], f32, tag="mm")
                        for kh in range(MH):
                            nc.tensor.matmul(
                                po[:], lhsT=w2sb[:, kh * P:(kh + 1) * P],
                                rhs=ht[kh][:, ns],
                                start=(kh == 0), stop=(kh == MH - 1))
                        grow = gtb[:, ns]
                        if e == 0:
                            nc.vector.tensor_tensor(
                                out=acc[om][:, ns], in0=po[:], in1=grow,
                                op=ALU.mult)
                        else:
                            tmp = tmpp.tile([P, NCH], f32, tag="sc", bufs=2)
                            nc.vector.tensor_tensor(
                                out=tmp[:], in0=po[:], in1=grow, op=ALU.mult)
                            nc.vector.tensor_add(
                                out=acc[om][:, ns], in0=acc[om][:, ns],
                                in1=tmp[:])

            # ---- bias2, transpose back to [token, feature], store ----
            for om in range(MO):
                nc.vector.tensor_tensor(
                    out=acc[om][:], in0=acc[om][:],
                    in1=b2sb[:, om:om + 1].to_broadcast([P, NTOK]),
                    op=ALU.add)
            for tm in range(TM):
                osb = outp.tile([P, O], f32, tag="osb", bufs=1)
                for om in range(MO):
                    ptt = pstr.tile([P, P], f32, tag="tr", name="ptt")
                    nc.tensor.transpose(
                        out=ptt[:], in_=acc[om][:, tm * P:(tm + 1) * P],
                        identity=idn[:])
                    nc.vector.tensor_copy(
                        out=osb[:, om * P:(om + 1) * P], in_=ptt[:])
                nc.sync.dma_start(
                    out=y[tm * P:(tm + 1) * P, :], in_=osb[:])

    nc.compile()
    return nc


def kernel(x, num_experts_chosen, W_gate, b_gate, W1, b1, W2, b2):
    assert int(num_experts_chosen) == 2
    x = np.ascontiguousarray(np.asarray(x, dtype=np.float32))
    W_gate = np.ascontiguousarray(np.asarray(W_gate, dtype=np.float32))
    b_gate = np.asarray(b_gate, dtype=np.float32)
    W1 = np.ascontiguousarray(np.asarray(W1, dtype=np.float32))
    b1 = np.asarray(b1, dtype=np.float32)
    W2 = np.ascontiguousarray(np.asarray(W2, dtype=np.float32))
    b2 = np.asarray(b2, dtype=np.float32)

    if "nc" not in _CACHE:
        _CACHE["nc"] = _build()
    nc = _CACHE["nc"]

    xtok = x.reshape(B * T, D)
    bgb = np.ascontiguousarray(np.broadcast_to(b_gate[None, :], (P, E)))
    b1c = np.ascontiguousarray(b1[:, :, None])
    b2c = np.ascontiguousarray(b2[:, None])
    in_maps = []
    for c in range(N_CORES):
        xs = np.ascontiguousarray(xtok[c * NTOK:(c + 1) * NTOK, :].T)
        in_maps.append({
            "xt": xs, "xtg": xs, "wg": W_gate, "bgb": bgb,
            "w1": W1, "b1": b1c, "w2": W2, "b2": b2c,
        })
    res = run_bass_kernel_spmd(nc, in_maps, core_ids=list(range(N_CORES)))
    out = np.concatenate([res.results[c]["y"] for c in range(N_CORES)], axis=0)
    return out.reshape(B, T, O)
